# revision 1
# baseline (speedup 1.0000x reference)
"""Distributed Bass kernel for causal multi-head attention with RoPE.

Problem: B=2, S=2048, D=2048, H=16, HD=128 (nn_Attention_85315230368481).

Sharding: head-parallel (tensor-parallel over heads). Core c owns heads
{2c, 2c+1}. x is replicated (sent transposed once to every core); each
core projects Q/K/V for only its two heads over the FULL sequence (same
FLOPs as projecting all heads for 1/8 of rows), applies RoPE, and
computes causal attention for its heads with block-level skipping (the
upper triangle is never computed -> ~53% of dense attention FLOPs,
identical on every core). The attention outputs are then redistributed
from head-sharding to row-sharding by two small AllToAll collectives
(1MB bf16 each; one per local head, so the first overlaps with the
second head's attention), and each core computes the output projection
for its 512 rows. No K/V AllGather (the v1 sequence-parallel baseline
spent ~750us of serialized Pool-engine time on those).

Pipeline/engine layout:
 - Wq/Wk columns permuted per head (even dims then odd dims) so RoPE
   operates on contiguous partition halves; scores are invariant.
 - RoPE as out = tcp*[cos;cos] + swap(tcp)*[-sin;sin]: every tensor op
   reads both inputs at the same base partition (HW verifier rule);
   the partition swap is two tiny SBUF-SBUF DMAs. Q-rope multiplies on
   DVE, K-rope multiplies on Pool.
 - Scores computed transposed ([keys, queries]) so exp(scores) is
   directly the moving operand of the attention*V matmul; exp runs on
   [128,1024] PSUM pairs (two key blocks per activation op).
 - Causal mask: 0/1 multiply after exp, only on the 4 diagonal
   relative offsets (off-diagonal blocks need no mask at all).
 - Softmax denominator: for head 0 accumulated on the otherwise-idle
   Pool engine (tensor_add + partition_all_reduce, SBUF only); for
   head 1 (when Pool runs the first AllToAll) an accumulating
   ones-column matmul on the PE.
 - Output projection: even heads arrive with AllToAll #1, so ALL
   even-head contributions are computed while AllToAll #2 is in
   flight, spilled to SBUF; after #2 lands only the odd-head
   accumulation remains and the drain is a PSUM+SBUF add.
 - All matmuls bf16 (f32 accumulation in PSUM).

Measured (timing-only CoreSim cost model, core 0): 349,874 ns vs
1,156,418 ns for the v1 baseline. Verified on TRN2 hardware via PJRT:
relative error 5.53e-03 (tolerance 2e-2).
"""

import sys

import ml_dtypes
import numpy as np

if "/opt/trn_rl_repo" not in sys.path:
    sys.path.insert(0, "/opt/trn_rl_repo")

B, S, D, H = 2, 2048, 2048, 16
HD = D // H            # 128
NCORES = 8
HL = 2                 # heads per core
R = B * S              # 4096 rows (row index = b*S + s)
RC = R // NCORES       # 512 rows per core in the output row-sharding
DCH = D // 128         # 16 contraction chunks
NQT = S // 512         # 4 query tiles of 512 per batch
SCALE = 1.0 / float(np.sqrt(HD))
BF16 = ml_dtypes.bfloat16

_GRAPH = None
_TRACE = False
_LAST_EXEC_NS = None
_LAST_RES = None


def _build_graph():
    import concourse.mybir as mybir
    from concourse import bacc, tile

    f32 = mybir.dt.float32
    bf = mybir.dt.bfloat16
    Exp = mybir.ActivationFunctionType.Exp

    nc = bacc.Bacc("TRN2", target_bir_lowering=False, num_devices=NCORES)

    xT = nc.declare_dram_parameter("xT", [D, R], bf, isOutput=False)
    wq = nc.declare_dram_parameter("wq", [D, HL * HD], bf, isOutput=False)
    wk = nc.declare_dram_parameter("wk", [D, HL * HD], bf, isOutput=False)
    wv = nc.declare_dram_parameter("wv", [D, HL * HD], bf, isOutput=False)
    wo = nc.declare_dram_parameter("wo", [D, D], bf, isOutput=False)
    cos2d = nc.declare_dram_parameter("cos2", [128, S], f32, isOutput=False)
    sin2d = nc.declare_dram_parameter("sin2m", [128, S], f32, isOutput=False)
    bmaskd = nc.declare_dram_parameter("bmask", [128, 4 * 512], bf, isOutput=False)
    onesd = nc.declare_dram_parameter("ones", [128, 128], bf, isOutput=False)
    out = nc.declare_dram_parameter("out", [RC, D], f32, isOutput=True)

    with nc.allow_low_precision(reason="bf16 matmul inputs; fp32 accumulate"), \
         tile.TileContext(nc) as tc:
        with (
            tc.tile_pool(name="dram", bufs=1, space="DRAM") as dramp,
            tc.tile_pool(name="const", bufs=1) as constp,
            tc.tile_pool(name="wop", bufs=24) as wop,
        ):
            a2a_in = [dramp.tile([NCORES * 128, RC], bf, name=f"a2a_in{l}")
                      for l in range(HL)]
            a2a_out = [dramp.tile([NCORES * 128, RC], bf, name=f"a2a_out{l}")
                       for l in range(HL)]

            # persistent SBUF
            qsb = constp.tile([128, HL * R], bf, name="qsb")      # rope'd Q^T per head
            ksb = constp.tile([128, HL * R], bf, name="ksb")      # rope'd K^T per head
            vsb = constp.tile([128, (R // 128) * 256], bf, name="vsb")  # V natural, per 128-row block
            # cos2 = [cos; cos], sin2m = [-sin; sin]: rope becomes
            # out = t * cos2 + swap(t) * sin2m with every tensor op reading
            # both inputs at the same base partition (HW constraint).
            cos_sb = constp.tile([128, S], f32, name="cos_sb")
            sin_sb = constp.tile([128, S], f32, name="sin_sb")
            bm_sb = constp.tile([128, 4 * 512], bf, name="bm_sb")
            ones_sb = constp.tile([128, 128], bf, name="ones_sb")
            asb = constp.tile([128, H * 512], bf, name="asb")
            wv_sb_fwd = constp.tile([128, DCH * 256], bf, name="wv_sb_fwd")
            # O-proj weight tiles in phase-3 consumption order; the first 24
            # (all even heads) prefetch on SP before the head-1 loop.
            wot_specs = [(2 * hh + par, g * 2 + nl)
                         for par in range(2) for g in range(2)
                         for nl in range(2) for hh in range(8)]
            wot_pre = []
            # wv on the Pool queue ahead of the trig tables: halves the ACT
            # weight backlog (rope tcp copies sit behind it), and the rope
            # multiplies that need cos/sin run long before anyone reads
            # their output.
            for dc in range(DCH):
                nc.gpsimd.dma_start(
                    out=wv_sb_fwd[:, dc * 256:(dc + 1) * 256],
                    in_=wv[dc * 128:(dc + 1) * 128, :],
                )
            nc.gpsimd.dma_start(out=cos_sb[:], in_=cos2d[:, :])
            nc.gpsimd.dma_start(out=sin_sb[:], in_=sin2d[:, :])
            nc.gpsimd.dma_start(out=bm_sb[:], in_=bmaskd[:, :])
            nc.gpsimd.dma_start(out=ones_sb[:], in_=onesd[:, :])

            # ---- Phase 1: Q/K/V projections for this core's 2 heads ----
            with (
                tc.tile_pool(name="wqkv", bufs=1) as wqkvp,
                tc.tile_pool(name="xts", bufs=17) as xtp,
                tc.tile_pool(name="qkps", bufs=1, space="PSUM") as qkps,
                tc.tile_pool(name="vps", bufs=4, space="PSUM") as vpsp,
                tc.tile_pool(name="ropetmp", bufs=2) as ropep,
            ):
                wq_sb = wqkvp.tile([128, DCH * 256], bf, name="wq_sb")
                wk_sb = wqkvp.tile([128, DCH * 256], bf, name="wk_sb")
                wv_sb = wv_sb_fwd
                for dc in range(DCH):
                    for w_sb, w in ((wq_sb, wq), (wk_sb, wk)):
                        nc.scalar.dma_start(
                            out=w_sb[:, dc * 256:(dc + 1) * 256],
                            in_=w[dc * 128:(dc + 1) * 128, :],
                        )

                def rope(eng, dma_eng, dst, dst_cols, src_ps, cs):
                    # src_ps: [128, 512] PSUM; rows 0:64 = even dims (te),
                    # 64:128 = odd dims (to).
                    #   re = te*cos - to*sin   -> dst[0:64]
                    #   im = te*sin + to*cos   -> dst[64:128]
                    # tcp = [te; to] (SBUF copy), tsw = [to; te] (partition
                    # swap via 2 DMAs); then dst = tcp*[cos;cos] +
                    # tsw*[-sin;sin] -- all ops same-base-partition.
                    dcs = slice(dst_cols, dst_cols + 512)
                    tcp = ropep.tile([128, 512], bf, tag="tcp", name="tcp")
                    nc.scalar.copy(tcp[:], src_ps[:, :])
                    tsw = ropep.tile([128, 512], bf, tag="tsw", name="tsw")
                    dma_eng.dma_start(out=tsw[0:64, :], in_=tcp[64:128, :])
                    dma_eng.dma_start(out=tsw[64:128, :], in_=tcp[0:64, :])
                    t1 = ropep.tile([128, 512], f32, tag="rt1", name="rt1")
                    t2 = ropep.tile([128, 512], f32, tag="rt2", name="rt2")
                    eng.tensor_mul(t1[:], tcp[:, :], cos_sb[:, cs])
                    eng.tensor_mul(t2[:], tsw[:, :], sin_sb[:, cs])
                    eng.tensor_add(dst[:, dcs], t1[:], t2[:])

                for rr in range(R // 512):          # 8 row chunks of 512
                    sq0 = (rr % 4) * 512            # within-batch offset
                    cs = slice(sq0, sq0 + 512)
                    qps = [qkps.tile([128, 512], f32, tag=f"qps{lh}", name=f"qps{lh}") for lh in range(HL)]
                    kps = [qkps.tile([128, 512], f32, tag=f"kps{lh}", name=f"kps{lh}") for lh in range(HL)]
                    vps = [vpsp.tile([128, 256], f32, tag="vps", name="vps") for _ in range(4)]
                    xts = []
                    for dc in range(DCH):
                        xt = xtp.tile([128, 512], bf, tag="xt", name="xt")
                        nc.sync.dma_start(
                            out=xt[:],
                            in_=xT[dc * 128:(dc + 1) * 128, rr * 512:(rr + 1) * 512],
                        )
                        xts.append(xt)
                        for lh in range(HL):
                            nc.tensor.matmul(
                                qps[lh][:],
                                lhsT=wq_sb[:, dc * 256 + lh * 128:dc * 256 + (lh + 1) * 128],
                                rhs=xt[:],
                                start=(dc == 0), stop=(dc == DCH - 1),
                            )
                            nc.tensor.matmul(
                                kps[lh][:],
                                lhsT=wk_sb[:, dc * 256 + lh * 128:dc * 256 + (lh + 1) * 128],
                                rhs=xt[:],
                                start=(dc == 0), stop=(dc == DCH - 1),
                            )
                    # V after Q/K: its matmuls keep the PE busy while the
                    # final Q/K rope chains drain.
                    for dc in range(DCH):
                        for rb2 in range(4):
                            nc.tensor.matmul(
                                vps[rb2][:],
                                lhsT=xts[dc][:, rb2 * 128:(rb2 + 1) * 128],
                                rhs=wv_sb[:, dc * 256:(dc + 1) * 256],
                                start=(dc == 0), stop=(dc == DCH - 1),
                            )
                    for lh in range(HL):
                        rope(nc.vector, nc.sync, qsb, lh * R + rr * 512, qps[lh], cs)
                        rope(nc.gpsimd, nc.scalar, ksb, lh * R + rr * 512, kps[lh], cs)
                    for rb2 in range(4):
                        kb32 = rr * 4 + rb2
                        nc.scalar.copy(
                            vsb[:, kb32 * 256:(kb32 + 1) * 256], vps[rb2][:]
                        )

            # ---- Phase 2: causal attention per head ----
            with (
                tc.tile_pool(name="scps", bufs=2, space="PSUM") as scps,
                tc.tile_pool(name="attps", bufs=2, space="PSUM") as attps,
                tc.tile_pool(name="smps", bufs=1, space="PSUM") as smps,
                tc.tile_pool(name="extiles", bufs=6) as expool,
                tc.tile_pool(name="tmp", bufs=2) as tmpp,
                tc.tile_pool(name="denacc", bufs=2) as denaccp,
                tc.tile_pool(name="aout", bufs=4) as aoutp,
            ):
                from concourse import bass_isa as _bisa
                def rescale(att, den, lh, b, qt):
                    rbs = tmpp.tile([128, 512], f32, tag="rbs", name="rbs")
                    if lh == 0:
                        # den: SBUF [128, 512], already partition-broadcast
                        nc.vector.reciprocal_approx_fast(rbs[:], den[:])
                    else:
                        # den: PSUM [1, 512] from the ones-matmul
                        dsb = tmpp.tile([1, 512], bf, tag="dsb", name="dsb")
                        nc.scalar.copy(dsb[:], den[:])
                        rb = smps.tile([128, 512], f32, tag="rb", name="rb")
                        nc.tensor.matmul(
                            rb[:], lhsT=ones_sb[0:1, :], rhs=dsb[:],
                            start=True, stop=True,
                        )
                        nc.vector.reciprocal_approx_fast(rbs[:], rb[:])
                    atile = aoutp.tile([128, 512], bf, tag="atile", name="atile")
                    nc.vector.tensor_mul(atile[:], att[:], rbs[:])
                    r = b * 4 + qt
                    nc.sync.dma_start(
                        out=a2a_in[lh][r * 128:(r + 1) * 128, :],
                        in_=atile[:],
                    )

                import concourse.mybir as mybir2
                pending = None
                for lh in range(HL):
                    for b in range(B):
                        for qt in range(NQT):
                            att = attps.tile([128, 512], f32, tag="att", name="att")
                            if lh == 0:
                                exsum = denaccp.tile([128, 512], f32,
                                                     tag="exsum", name="exsum")
                            else:
                                den = smps.tile([1, 512], f32, tag="den", name="den")
                            nkb = 4 * qt + 4
                            kcs0 = lh * R + b * S
                            qcs = slice(kcs0 + qt * 512, kcs0 + (qt + 1) * 512)
                            prev = []
                            for kp in range(nkb // 2):
                                if kp == 1 and pending is not None:
                                    rescale(*pending)
                                    pending = None
                                scp = scps.tile([128, 1024], f32, tag="scp", name="scp")
                                for jj in range(2):
                                    kb = 2 * kp + jj
                                    nc.tensor.matmul(
                                        scp[:, jj * 512:(jj + 1) * 512],
                                        lhsT=ksb[:, kcs0 + kb * 128:kcs0 + (kb + 1) * 128],
                                        rhs=qsb[:, qcs],
                                        start=True, stop=True,
                                    )
                                ex2 = expool.tile([128, 1024], bf, tag="ex", name="ex")
                                nc.scalar.activation(ex2[:], scp[:], Exp, scale=SCALE)
                                cur = []
                                for jj in range(2):
                                    kb = 2 * kp + jj
                                    o = max(kb - 4 * qt, 0)
                                    q0 = 0
                                    e = ex2[:, jj * 512:(jj + 1) * 512]
                                    if kb >= 4 * qt:
                                        exm = expool.tile([128, 512], bf, tag="exm", name="exm")
                                        nc.vector.tensor_mul(
                                            exm[:], e, bm_sb[:, o * 512:(o + 1) * 512]
                                        )
                                        e = exm[:]
                                    if lh == 0:
                                        if kb == 0:
                                            nc.gpsimd.tensor_scalar_mul(
                                                exsum[:], e, 1.0)
                                        else:
                                            nc.gpsimd.tensor_add(
                                                exsum[:], exsum[:], e)
                                    cur.append((e, kb, q0))
                                for pex, pkb, pq0 in prev:
                                    if lh == 1:
                                        nc.tensor.matmul(
                                            den[:, pq0:512], lhsT=ones_sb[:, 0:1],
                                            rhs=pex,
                                            start=(pkb == 0), stop=False,
                                        )
                                    nc.tensor.matmul(
                                        att[:, pq0:512],
                                        lhsT=vsb[:, (b * 16 + pkb) * 256 + lh * 128:
                                                 (b * 16 + pkb) * 256 + (lh + 1) * 128],
                                        rhs=pex,
                                        start=(pkb == 0), stop=False,
                                    )
                                prev = cur
                            for pex, pkb, pq0 in prev:
                                if lh == 1:
                                    nc.tensor.matmul(
                                        den[:, pq0:512], lhsT=ones_sb[:, 0:1],
                                        rhs=pex,
                                        start=(pkb == 0), stop=(pkb == nkb - 1),
                                    )
                                nc.tensor.matmul(
                                    att[:, pq0:512],
                                    lhsT=vsb[:, (b * 16 + pkb) * 256 + lh * 128:
                                             (b * 16 + pkb) * 256 + (lh + 1) * 128],
                                    rhs=pex,
                                    start=(pkb == 0), stop=(pkb == nkb - 1),
                                )
                            if lh == 0:
                                densb = denaccp.tile([128, 512], f32,
                                                     tag="densb", name="densb")
                                nc.gpsimd.partition_all_reduce(
                                    densb[:], exsum[:], channels=128,
                                    reduce_op=_bisa.ReduceOp.add,
                                )
                                pending = (att, densb, lh, b, qt)
                            else:
                                pending = (att, den, lh, b, qt)
                    # flush this head's last tile, then exchange it:
                    # a2a(lh=0) overlaps with lh=1 attention.
                    rescale(*pending)
                    pending = None
                    nc.gpsimd.collective_compute(
                        "AllToAll",
                        mybir2.AluOpType.bypass,
                        replica_groups=[list(range(NCORES))],
                        ins=[a2a_in[lh].opt()],
                        outs=[a2a_out[lh].opt()],
                    )
                    if lh == 1:
                        # odd-head asb loads: same trick as the evens --
                        # Pool issues them the moment a2a #2 completes.
                        for h in range(1, H, 2):
                            nc.gpsimd.dma_start(
                                out=asb[:, h * 512:(h + 1) * 512],
                                in_=a2a_out[1][(h // 2) * 128:(h // 2 + 1) * 128, :],
                            )
                    if lh == 0:
                        # even-head asb loads run on the Pool queue the
                        # moment a2a #1 completes; wot prefetch uses the
                        # otherwise-idle SP queue during head-1 attention.
                        for h in range(0, H, 2):
                            nc.gpsimd.dma_start(
                                out=asb[:, h * 512:(h + 1) * 512],
                                in_=a2a_out[0][(h // 2) * 128:(h // 2 + 1) * 128, :],
                            )
                        for h, nn in wot_specs[:24]:
                            wt = wop.tile([128, 512], bf, tag="wot", name="wot")
                            nc.sync.dma_start(
                                out=wt[:],
                                in_=wo[h * 128:(h + 1) * 128,
                                       nn * 512:(nn + 1) * 512],
                            )
                            wot_pre.append(wt)

            # ---- Phase 3: output projection for this core's 512 rows ----
            # Even heads arrive with a2a #1 (early): compute ALL even-head
            # contributions while a2a #2 is in flight, spilling each PSUM
            # group to SBUF. After a2a #2 only the odd-head accumulation
            # remains; the drain is a PSUM+SBUF add.
            with (
                tc.tile_pool(name="ops", bufs=1, space="PSUM") as opsp,
                tc.tile_pool(name="esbp", bufs=1) as esbp,
                tc.tile_pool(name="osb", bufs=4) as osbp,
            ):
                esb = esbp.tile([128, 16 * 512], f32, name="esb")
                for par in range(2):
                    for g in range(2):          # groups of 2 output-col blocks
                        pt = {}
                        for nl in range(2):
                            for qt2 in range(4):
                                pt[(nl, qt2)] = opsp.tile(
                                    [128, 512], f32, tag=f"o{nl}{qt2}", name="ops",
                                )
                        for nl in range(2):
                            nn = g * 2 + nl
                            for hh in range(8):
                                h = 2 * hh + par
                                widx = wot_specs.index((h, nn))
                                if widx < 24:
                                    wot = wot_pre[widx]
                                else:
                                    wot = wop.tile([128, 512], bf,
                                                   tag="wot", name="wot")
                                    nc.sync.dma_start(
                                        out=wot[:],
                                        in_=wo[h * 128:(h + 1) * 128,
                                               nn * 512:(nn + 1) * 512],
                                    )
                                for qt2 in range(4):
                                    nc.tensor.matmul(
                                        pt[(nl, qt2)][:],
                                        lhsT=asb[:, h * 512 + qt2 * 128:
                                                 h * 512 + (qt2 + 1) * 128],
                                        rhs=wot[:],
                                        start=(hh == 0), stop=(hh == 7),
                                    )
                        for nl in range(2):
                            nn = g * 2 + nl
                            for qt2 in range(4):
                                es = esb[:, (nn * 4 + qt2) * 512:
                                          (nn * 4 + qt2 + 1) * 512]
                                if par == 0:
                                    # spill even-head partials to SBUF
                                    if qt2 % 2 == 0:
                                        nc.scalar.copy(es, pt[(nl, qt2)][:])
                                    else:
                                        nc.vector.tensor_scalar_mul(
                                            es, pt[(nl, qt2)][:], 1.0)
                                else:
                                    # combine odd-head PSUM with even spill
                                    osb = osbp.tile([128, 512], f32,
                                                    tag="osb", name="osb")
                                    nc.vector.tensor_add(
                                        osb[:], pt[(nl, qt2)][:], es)
                                    nc.sync.dma_start(
                                        out=out[qt2 * 128:(qt2 + 1) * 128,
                                                nn * 512:(nn + 1) * 512],
                                        in_=osb[:],
                                    )

    nc.compile()
    return nc


def _get_graph():
    global _GRAPH
    if _GRAPH is None:
        _GRAPH = _build_graph()
    return _GRAPH


# per-head column permutation: even dims then odd dims
_EO = np.concatenate([np.arange(0, HD, 2), np.arange(1, HD, 2)])


def kernel(x, Wq, Wk, Wv, Wo, freqs_cos, freqs_sin, mask):
    global _LAST_EXEC_NS, _LAST_RES
    from concourse.bass_utils import run_bass_kernel_spmd

    nc = _get_graph()

    x = np.asarray(x, np.float32)
    xT = np.ascontiguousarray(x.reshape(R, D).T).astype(BF16)
    wo_b = np.ascontiguousarray(np.asarray(Wo, np.float32)).astype(BF16)
    cosT_ = np.asarray(freqs_cos, np.float32).T            # [64, S]
    sinT_ = np.asarray(freqs_sin, np.float32).T
    cos2 = np.ascontiguousarray(np.concatenate([cosT_, cosT_], axis=0))
    sin2m = np.ascontiguousarray(np.concatenate([-sinT_, sinT_], axis=0))

    # 0/1 relative diagonal masks from the provided additive mask:
    # bm[o][k, q] = 1 iff query q may attend key 128*o + k (block-relative).
    maskf = np.asarray(mask, np.float32)[0, 0]
    bm = np.empty((128, 4 * 512), np.float32)
    for o in range(4):
        bm[:, o * 512:(o + 1) * 512] = (
            maskf[:512, o * 128:(o + 1) * 128] > -0.5
        ).T.astype(np.float32)
    bm = bm.astype(BF16)
    ones_b = np.ones((128, 128), BF16)

    Wqf = np.asarray(Wq, np.float32)
    Wkf = np.asarray(Wk, np.float32)
    Wvf = np.asarray(Wv, np.float32)

    in_maps = []
    for c in range(NCORES):
        h0, h1 = 2 * c, 2 * c + 1
        pcols = np.concatenate([h0 * HD + _EO, h1 * HD + _EO])
        ncols = np.concatenate(
            [np.arange(h0 * HD, (h0 + 1) * HD), np.arange(h1 * HD, (h1 + 1) * HD)]
        )
        in_maps.append({
            "xT": xT,
            "wq": np.ascontiguousarray(Wqf[:, pcols]).astype(BF16),
            "wk": np.ascontiguousarray(Wkf[:, pcols]).astype(BF16),
            "wv": np.ascontiguousarray(Wvf[:, ncols]).astype(BF16),
            "wo": wo_b,
            "cos2": cos2,
            "sin2m": sin2m,
            "bmask": bm,
            "ones": ones_b,
        })

    res = run_bass_kernel_spmd(
        nc, in_maps, core_ids=list(range(NCORES)), trace=_TRACE,
    )
    _LAST_EXEC_NS = res.exec_time_ns
    _LAST_RES = res

    outp = np.empty((R, D), np.float32)
    for c in range(NCORES):
        outp[c * RC:(c + 1) * RC, :] = res.results[c]["out"]
    return outp.reshape(B, S, D)

